# revision 63
# baseline (speedup 1.0000x reference)
"""Trainium2 Bass kernel for causal multi-head attention with QKV/O projections.

Problem: x [1, 2048, 1024] f32, W_qkv [1024, 3072] (q|k|v blocks), W_o
[1024, 1024], H=16 heads, head_dim=64, dense causal attention,
y = softmax(q k^T / 8, causal) v, out = y @ W_o.

Sharding: head-parallel over 8 NeuronCores (2 heads per core). Each core
computes q/k/v projections for its 2 heads, causal attention, and a partial
O-projection (its 128 attention-output columns against its 128 rows of W_o).
The host sums the 8 partial outputs.

On-core dataflow (bf16 into the PE, f32 accumulation in PSUM):
  - xT [128, (r d c)] arrives pre-transposed and pre-blocked from the host
    (column block r, contraction chunk d), so the r=0 quarter lands first
    and projections start before the rest of x arrives.
  - all weights + mask + identity arrive in ONE dram tensor w_all
    [128, 4352] = wq|wk|wv|wo|mask|ident (one DMA issue instead of six).
  - qT/kT/vT [128, T] = W.T @ xT (2 heads stacked on partitions); v is then
    materialized in [tk, hd] layout via PE transposes of vT 128x128 blocks
    (bf16 transpose datapath), evacuated by GpSimd into v_sb with a
    constant-1 column prepended per head ([1 | v_h]), so the attention-V
    matmul also accumulates the softmax denominator at partition 0.
  - attention is computed transposed: S_T [tk, tq] = kT-tile.T @ qT-tile,
    both heads concurrently via PE tiling (K=64 row halves), P_T = exp(S/8)
    in one ACT op per (tk, tq-block) covering both heads, causal mask
    applied on diagonal 128x128 blocks by a DVE multiply; fully-masked
    blocks skipped and both heads column-trimmed on diagonal blocks.
  - den/numer_T: [65, tq] = [1 | v_h].T @ P_T per head (den at row 0).
  - normalize: reciprocal_approx_fast directly on the PSUM den row (base
    0), cast bf16, broadcast to 128 partitions with a K=1 PE matmul, one
    DVE multiply per head writes normalized att rows.
  - y_partial [T, D] = att.T.T @ wo_rows; PSUM evacuated by DVE (cols
    0:512) and GpSimd (cols 512:1024), DMA'd bf16 on the Sync engine;
    summed on the host.

Engine budget: ACT does exp only (its table is preloaded during the input
DMA); DVE does casts/mask/normalize + half the O-proj evac; GpSimd does the
v scatter + the other half of the O-proj evac; Sync issues all DMAs.

Scheduling: the PE has a p-state ramp (full 2.4 GHz only after ~3us of
continuous execution), so the emission order keeps the PE dense:
  - warm-up matmuls on memset scratch cover the input-DMA wait;
  - the attention i-loop is software-pipelined: S(i+1) is emitted before
    AV(i); projection/O-projection work is drained quota-wise inside the
    i-loop to fill PE slack under the ACT exp stream;
  - O-proj tiles are spread across round boundaries so the post-round-3
    tail is only normalize(3) + 4 tiles.
"""

from contextlib import ExitStack

import numpy as np
import ml_dtypes

import concourse.bacc as bacc
import concourse.mybir as mybir
import concourse.tile as tile

BF16 = ml_dtypes.bfloat16
T = 2048
D = 1024
HD = 64
N_CORES = 8
KD = D // 128          # 8 contraction chunks for projections
NT128 = T // 128       # 16
NT512 = T // 512       # 4
VS = 256               # v_sb per-tile stride: [1*64 | v_h0(64) | 1*64 | v_h1(64)]
SCALE = 1.0 / 8.0      # 1/sqrt(64)
WARM_N = 36            # p-state warm-up matmuls while input DMA is in flight

# w_all column offsets
WQ0, WK0, WV0, WO0, MSK0, ID0 = 0, D, 2 * D, 3 * D, 4 * D, 4 * D + 128
WCOLS = 4 * D + 256

F32 = mybir.dt.float32
BF = mybir.dt.bfloat16

_SENTINEL = object()


class _Work:
    """Wraps an emission generator that yields False mid-segment and True at
    segment boundaries (points where every pool accumulation it opened is
    closed, so other users of the same pools may allocate)."""

    def __init__(self, gen):
        self.gen = gen
        self.at_boundary = True
        self.done = False

    def step(self):
        r = next(self.gen, _SENTINEL)
        if r is _SENTINEL:
            self.done = True
            self.at_boundary = True
            return False
        self.at_boundary = bool(r)
        return True

    def drain_to_boundary(self):
        while not (self.at_boundary or self.done):
            self.step()

    def drain_all(self):
        while not self.done:
            self.step()


def _kernel(tc, y, xT, w_all, dbg=None):
    nc = tc.nc
    Exp = mybir.ActivationFunctionType.Exp

    with ExitStack() as ctx:
        persist = ctx.enter_context(tc.tile_pool(name="persist", bufs=1))
        ps_mm = ctx.enter_context(tc.tile_pool(name="ps_mm", bufs=2, space="PSUM"))
        ps_s = ctx.enter_context(tc.tile_pool(name="ps_s", bufs=2, space="PSUM"))
        ps_av = ctx.enter_context(tc.tile_pool(name="ps_av", bufs=1, space="PSUM"))
        pool_p = ctx.enter_context(tc.tile_pool(name="pool_p", bufs=5))
        pool_r = ctx.enter_context(tc.tile_pool(name="pool_r", bufs=2))
        pool_y = ctx.enter_context(tc.tile_pool(name="pool_y", bufs=6))

        w_sb = persist.tile([128, WCOLS], BF, tag="w")
        xT_sb = persist.tile([128, KD * T], BF, tag="xT")  # block (r,d) at (r*8+d)*512

        # ---- p-state warm-up scratch.
        warm_sb = persist.tile([128, 640], BF, tag="warm")
        nc.vector.memset(warm_sb[:], 0.5)

        # ---- input DMA: qkv weights first, then each xT quarter striped
        # across the three issue queues IN QUARTER ORDER (sync carries the
        # w tensors, so its stripe is smaller), so the DMA engines finish
        # r0 before starting r1 and projections start early.
        nc.sync.dma_start(w_sb[:, 0:3 * D], w_all[:, 0:3 * D])
        act_warm = persist.tile([1, 8], F32, tag="actwarm")
        for r in range(4):
            q0 = r * 4096
            nc.scalar.dma_start(xT_sb[:, q0:q0 + 1792], xT[:, q0:q0 + 1792])
            nc.gpsimd.dma_start(
                xT_sb[:, q0 + 1792:q0 + 3584], xT[:, q0 + 1792:q0 + 3584])
            nc.sync.dma_start(
                xT_sb[:, q0 + 3584:q0 + 4096], xT[:, q0 + 3584:q0 + 4096])
            if r == 0:
                # wo|mask|ident after the r0 stripes (needed later than wqkv)
                nc.sync.dma_start(w_sb[:, 3 * D:WCOLS], w_all[:, 3 * D:WCOLS])
                # preload the ACT exp table during the DMA wait (the first
                # activation otherwise pays ~1.3us of table load on the
                # attention critical path)
                nc.scalar.activation(
                    act_warm[:], warm_sb[0:1, 0:8], Exp, scale=SCALE)

        qT_sb = persist.tile([128, T], BF, tag="qT")   # partitions 0-63 head0, 64-127 head1
        kT_sb = persist.tile([128, T], BF, tag="kT")
        vT_sb = persist.tile([128, T], BF, tag="vT")
        v_sb = persist.tile([128, NT128 * VS], BF, tag="v")
        v_cols = v_sb[:].rearrange("p (t s) -> p t s", s=VS)
        nc.vector.memset(v_cols[:, :, 0:64], 1.0)      # den-broadcast columns
        nc.vector.memset(v_cols[:, :, 128:192], 1.0)
        att_sb = persist.tile([128, T], BF, tag="att")  # normalized numer_T

        wq_sb = w_sb[:, WQ0:WQ0 + D]
        wk_sb = w_sb[:, WK0:WK0 + D]
        wv_sb = w_sb[:, WV0:WV0 + D]
        wo_sb = w_sb[:, WO0:WO0 + D]
        mask_sb = w_sb[:, MSK0:MSK0 + 128]
        id_sb = w_sb[:, ID0:ID0 + 128]

        def proj_gen(r):
            """QKV projections for column-block r: 24 N=512 matmuls + 3 casts
            + 4 transposes (+ GpSimd scatters). Yields True when the open
            ps_mm segment has been closed."""
            for w_part, dst in ((wq_sb, qT_sb), (wk_sb, kT_sb), (wv_sb, vT_sb)):
                ps = ps_mm.tile([128, 512], F32, tag="mm")
                for d in range(KD):
                    nc.tensor.matmul(
                        ps[:],
                        lhsT=w_part[:, d * 128:(d + 1) * 128],
                        rhs=xT_sb[:, (r * KD + d) * 512:(r * KD + d + 1) * 512],
                        start=(d == 0), stop=(d == KD - 1),
                    )
                    yield False
                nc.vector.tensor_copy(dst[:, r * 512:(r + 1) * 512], ps[:])
                yield True
            for t in range(4 * r, 4 * r + 4):
                ps_t = ps_mm.tile([128, 128], BF, tag="mm")
                nc.tensor.transpose(
                    ps_t[:], vT_sb[:, t * 128:(t + 1) * 128], id_sb)
                yield False
                dst = v_cols[:, t, :].rearrange(
                    "p (g q) -> p g q", q=128)[:, :, 64:128]
                src = ps_t[:].rearrange("p (g q) -> p g q", q=64)
                nc.vector.tensor_copy(dst, src)
                yield True

        def oproj_gen(tiles, dve_only=False, act_evac=False):
            """O-projection rows for the given T-chunk indices; evacuation
            split DVE (cols 0:512) / ACT (cols 512:1024) so neither engine
            builds a backlog that would delay the round-boundary normalize.
            dve_only keeps evacs off ACT when the round is exp-paced;
            act_evac puts both on ACT (tail: DVE runs normalize(3))."""
            for t in tiles:
                y_sb = pool_y.tile([128, 1024], BF, tag="y")
                ps0 = ps_mm.tile([128, 512], F32, tag="mm")
                nc.tensor.matmul(
                    ps0[:], lhsT=att_sb[:, t * 128:(t + 1) * 128],
                    rhs=wo_sb[:, 0:512], start=True, stop=True,
                )
                yield False
                ps1 = ps_mm.tile([128, 512], F32, tag="mm")
                nc.tensor.matmul(
                    ps1[:], lhsT=att_sb[:, t * 128:(t + 1) * 128],
                    rhs=wo_sb[:, 512:1024], start=True, stop=True,
                )
                yield False
                if act_evac:
                    nc.scalar.copy(y_sb[:, 0:512], ps0[:])
                else:
                    nc.vector.tensor_copy(y_sb[:, 0:512], ps0[:])
                yield True
                if dve_only:
                    nc.vector.tensor_copy(y_sb[:, 512:1024], ps1[:])
                else:
                    nc.scalar.copy(y_sb[:, 512:1024], ps1[:])
                yield True
                nc.sync.dma_start(y[t * 128:(t + 1) * 128, :], y_sb[:])
                yield True

        def oproj_tail_gen(tiles):
            """Tail O-projection: the S-pipeline PSUM banks are free, so use
            [128,1024] ps_s tiles (both matmuls in flight per tile, no
            ps_mm round-trip). The first half of the tiles evacuates on ACT
            alone (DVE is still running normalize(3)'s second chunk); the
            rest split DVE/ACT. Each half DMAs out as soon as it lands."""
            for idx, t in enumerate(tiles):
                y_sb = pool_y.tile([128, 1024], BF, tag="y")
                ps = ps_s.tile([128, 1024], F32, tag="s")
                nc.tensor.matmul(
                    ps[:, 0:512], lhsT=att_sb[:, t * 128:(t + 1) * 128],
                    rhs=wo_sb[:, 0:512], start=True, stop=True,
                )
                yield False
                nc.tensor.matmul(
                    ps[:, 512:1024], lhsT=att_sb[:, t * 128:(t + 1) * 128],
                    rhs=wo_sb[:, 512:1024], start=True, stop=True,
                )
                yield False
                if idx < len(tiles) // 2:
                    nc.scalar.copy(y_sb[:, 0:512], ps[:, 0:512])
                else:
                    nc.vector.tensor_copy(y_sb[:, 0:512], ps[:, 0:512])
                yield True
                eng = nc.gpsimd if t % 2 else nc.sync
                eng.dma_start(
                    y[t * 128:(t + 1) * 128, 0:512], y_sb[:, 0:512])
                nc.scalar.copy(y_sb[:, 512:1024], ps[:, 512:1024])
                yield True
                eng.dma_start(
                    y[t * 128:(t + 1) * 128, 512:1024], y_sb[:, 512:1024])
                yield True

        def S_pair(j, i):
            """S_T block matmuls for (tq round j, tk block i): both heads
            concurrently on disjoint PE row-halves (K=64 tile positions)."""
            m = i - 4 * j
            off = 128 * m if m > 0 else 0
            s_pair = ps_s.tile([128, 1024], F32, tag="s")
            nc.tensor.matmul(
                s_pair[:, off:512],
                lhsT=kT_sb[0:64, i * 128:(i + 1) * 128],
                rhs=qT_sb[0:64, j * 512 + off:(j + 1) * 512],
                start=True, stop=True, tile_position=(0, 0),
            )
            nc.tensor.matmul(
                s_pair[:, 512:1024 - off],
                lhsT=kT_sb[64:128, i * 128:(i + 1) * 128],
                rhs=qT_sb[64:128, j * 512 + off:(j + 1) * 512],
                start=True, stop=True, tile_position=(64, 0),
            )
            return s_pair

        def attn_round(j, work, total, reserve, s0=None):
            """Causal attention for tq block j, software-pipelined; spreads
            ~(total - reserve) interleave items evenly over the i-loop so the
            PE stays ahead of ACT on every iteration. s0 is the pre-emitted
            S-pair for i=0 (emitted before the previous round boundary's
            PSUM-gated filler, so the PE never goes idle at the boundary)."""
            n_i = 4 * j + 4
            remaining = max(0, total - reserve)
            avden = ps_av.tile([128, 1024], F32, tag="avden")
            s_tiles = [None] * n_i

            s_tiles[0] = s0 if s0 is not None else S_pair(j, 0)
            for i in range(n_i):
                if i + 1 < n_i:
                    s_tiles[i + 1] = S_pair(j, i + 1)
                m = i - 4 * j
                off = 128 * m if m > 0 else 0
                ncol = 512 - off
                first, last = (i == 0), (i == n_i - 1)
                s_pair = s_tiles[i]
                s_tiles[i] = None
                p_sb = pool_p.tile([128, 1024], BF, tag="p")
                nc.scalar.activation(
                    p_sb[:, off:512 + ncol], s_pair[:, off:512 + ncol], Exp,
                    scale=SCALE,
                )
                if m >= 0:  # causal mask on the 128x128 diagonal sub-blocks
                    nc.vector.tensor_mul(
                        p_sb[:, off:off + 128], p_sb[:, off:off + 128], mask_sb,
                    )
                    nc.vector.tensor_mul(
                        p_sb[:, 512:640], p_sb[:, 512:640], mask_sb,
                    )
                # interleave filler BEFORE the AV pair: the PE chews on it
                # while ACT finishes exp(i) (and the pool the mask), instead
                # of idling in-order behind the AV's data dependency.
                k = -(-remaining // (n_i - i))  # ceil: spread evenly
                remaining -= k
                for _ in range(k):
                    if not work.step():
                        break
                nc.tensor.matmul(
                    avden[:, off:512],
                    lhsT=v_sb[:, VS * i: VS * i + 128],
                    rhs=p_sb[:, off:512],
                    start=first, stop=last,
                )
                nc.tensor.matmul(
                    avden[:, 512 + off:1024],
                    lhsT=v_sb[:, VS * i + 128: VS * i + 256],
                    rhs=p_sb[:, 512:512 + ncol],
                    start=first, stop=last,
                )
            return avden

        def normalize(j, avden, split=1):
            """The AV matmul already broadcast the denominator to partitions
            0-63 (64 ones columns in the v lhsT); recip it straight from
            PSUM (base 0) and one DVE mul per head writes normalized att.
            split>1 processes tq column chunks separately so downstream
            O-proj tiles unlock as soon as their chunk is normalized."""
            w = 512 // split
            recs = {}
            for c in range(split):
                for h in range(2):
                    rec = pool_r.tile([64, w], F32, tag=f"rec{c}{h}")
                    nc.vector.reciprocal_approx_fast(
                        rec[:], avden[0:64, h * 512 + c * w:h * 512 + (c + 1) * w])
                    recs[c, h] = rec
            for c in range(split):
                for h in range(2):
                    nc.vector.tensor_mul(
                        att_sb[h * 64:(h + 1) * 64,
                               j * 512 + c * w:j * 512 + (c + 1) * w],
                        avden[64:128, h * 512 + c * w:h * 512 + (c + 1) * w],
                        recs[c, h][:],
                    )

        # warm-up stream, then round 0 projections (PE hot when xT lands)
        warm_ps = ps_s.tile([128, 1024], F32, tag="s")
        for _ in range(WARM_N):
            nc.tensor.matmul(
                warm_ps[:, 0:512], lhsT=warm_sb[:, 0:128], rhs=warm_sb[:, 128:640],
                start=True, stop=True,
            )
        for _ in proj_gen(0):
            pass

        # r=0: attn(0) ⟂ proj(1); normalize(0); proj(1) rest; oproj t 0-1
        # r=1: attn(1) ⟂ proj(2); normalize(1); proj(2) rest; oproj t 2-3
        # r=2: attn(2) ⟂ proj(3); normalize(2); proj(3) rest; oproj t 4-5
        # r=3: attn(3) ⟂ oproj t 6-7 (small: attn3 is ACT-paced);
        #      normalize(3) with oproj t 8-11 matmuls feeding the PE while
        #      the DVE runs the recip/mul chain; then the ps_s-based tail.
        work = _Work(proj_gen(1))
        s0 = None
        for r in range(NT512):
            if r < 3:
                avden = attn_round(r, work, 35, 6, s0=s0)
            else:
                avden = attn_round(r, work, 35, 4, s0=s0)
            # pre-emit the next round's first S-pair so the PE has
            # dependency-free work queued ahead of the PSUM-slot-gated
            # boundary fillers (in-order head-of-line blocking otherwise
            # idles the PE and drops its p-state clock)
            s0 = S_pair(r + 1, 0) if r + 1 < NT512 else None
            # normalize only touches pool_r/avden/att, so emit it before the
            # leftover work: its DVE ops jump ahead of the remaining evacs
            normalize(r, avden, split=(2 if r == 3 else 1))
            work.drain_all()
            if r == 0:
                _Work(oproj_gen([0, 1])).drain_all()
                work = _Work(proj_gen(2))
            elif r == 1:
                work = _Work(proj_gen(3))
            elif r == 2:
                work = _Work(
                    oproj_gen([2, 3, 4, 5, 6, 7, 8], dve_only=True))
            else:
                _Work(oproj_gen([9, 10, 11], act_evac=True)).drain_all()
                _Work(oproj_tail_gen([12, 13, 14, 15])).drain_all()

        if dbg is not None:
            for name, sb in (("qT", qT_sb), ("kT", kT_sb), ("vT", vT_sb),
                             ("att", att_sb)):
                nc.sync.dma_start(dbg[name][:], sb[:])
            nc.sync.dma_start(dbg["v"][:], v_sb[:])
            nc.sync.dma_start(dbg["xT"][:], xT_sb[:])
            nc.sync.dma_start(dbg["w"][:], w_sb[:])


def _build_program(debug_dumps=False):
    nc = bacc.Bacc("TRN2", debug=False, num_devices=N_CORES)
    xT = nc.dram_tensor("xT", [128, KD * T], BF, kind="ExternalInput").ap()
    w_all = nc.dram_tensor("w_all", [128, WCOLS], BF, kind="ExternalInput").ap()
    y = nc.dram_tensor("y", [T, D], BF, kind="ExternalOutput").ap()
    dbg = None
    if debug_dumps:
        dbg = {
            name: nc.dram_tensor(f"dbg_{name}", [128, T], BF, kind="ExternalOutput").ap()
            for name in ("qT", "kT", "vT", "att")
        }
        dbg["v"] = nc.dram_tensor("dbg_v", [128, NT128 * VS], BF, kind="ExternalOutput").ap()
        dbg["xT"] = nc.dram_tensor("dbg_xT", [128, KD * T], BF, kind="ExternalOutput").ap()
        dbg["w"] = nc.dram_tensor("dbg_w", [128, WCOLS], BF, kind="ExternalOutput").ap()

    with tile.TileContext(nc) as tc:
        _kernel(tc, y, xT, w_all, dbg=dbg)
    nc.compile()
    return nc


_NC = None


def _get_program():
    global _NC
    if _NC is None:
        _NC = _build_program()
    return _NC


def _rearrange_w(w_cols):
    """[1024, 128] f32 slice of W_qkv -> [128, 1024] bf16 with d-chunk d at
    cols [d*128, (d+1)*128): out[p, d*128 + m] = w_cols[d*128 + p, m]."""
    return np.ascontiguousarray(
        w_cols.reshape(KD, 128, 128).transpose(1, 0, 2).reshape(128, KD * 128)
    ).astype(BF16)


def make_in_maps(x, W_qkv, W_o):
    x2 = np.asarray(x, dtype=np.float32).reshape(T, D)
    W_qkv = np.asarray(W_qkv, dtype=np.float32)
    W_o = np.asarray(W_o, dtype=np.float32)

    # xT blocked [p, r, d, c]: col (r*8+d)*512 + c holds x[r*512+c, d*128+p]
    xT_bf = np.ascontiguousarray(
        x2.T.reshape(KD, 128, NT512, 512).transpose(1, 2, 0, 3).reshape(128, KD * T)
    ).astype(BF16)
    mask = np.triu(np.ones((128, 128), dtype=np.float32)).astype(BF16)
    ident = np.eye(128, dtype=np.float32).astype(BF16)

    in_maps = []
    for c in range(N_CORES):
        cs = slice(2 * c * HD, 2 * c * HD + 128)
        w_all = np.concatenate([
            _rearrange_w(W_qkv[:, 0 * D:1 * D][:, cs]),
            _rearrange_w(W_qkv[:, 1 * D:2 * D][:, cs]),
            _rearrange_w(W_qkv[:, 2 * D:3 * D][:, cs]),
            np.ascontiguousarray(W_o[c * 128:(c + 1) * 128, :]).astype(BF16),
            mask,
            ident,
        ], axis=1)
        in_maps.append({"xT": xT_bf, "w_all": w_all})
    return in_maps


def combine_outputs(results):
    y_full = np.zeros((T, D), dtype=np.float32)
    for c in range(N_CORES):
        y_full += results[c]["y"].astype(np.float32)
    return y_full.reshape(1, T, D)


def kernel(x, W_qkv, W_o):
    from concourse.bass_utils import run_bass_kernel_spmd

    nc = _get_program()
    in_maps = make_in_maps(x, W_qkv, W_o)
    res = run_bass_kernel_spmd(nc, in_maps, core_ids=list(range(N_CORES)))
    return combine_outputs(res.results)


# revision 64
# speedup vs baseline: 1.1810x; 1.1810x over previous
"""Trainium2 Bass kernel for causal multi-head attention with QKV/O projections.

Problem: x [1, 2048, 1024] f32, W_qkv [1024, 3072] (q|k|v blocks), W_o
[1024, 1024], H=16 heads, head_dim=64, dense causal attention,
y = softmax(q k^T / 8, causal) v, out = y @ W_o.

Sharding: head-parallel over 8 NeuronCores (2 heads per core). Each core
computes q/k/v projections for its 2 heads, causal attention, and a partial
O-projection (its 128 attention-output columns against its 128 rows of W_o).
The host sums the 8 partial outputs.

On-core dataflow (bf16 into the PE, f32 accumulation in PSUM):
  - xT [128, (r d c)] arrives pre-transposed and pre-blocked from the host
    (column block r, contraction chunk d), so the r=0 quarter lands first
    and projections start before the rest of x arrives.
  - all weights + mask + identity arrive in ONE dram tensor w_all
    [128, 4352] = wq|wk|wv|wo|mask|ident (one DMA issue instead of six).
  - qT/kT/vT [128, T] = W.T @ xT (2 heads stacked on partitions); v is then
    materialized in [tk, hd] layout via PE transposes of vT 128x128 blocks
    (bf16 transpose datapath), evacuated by GpSimd into v_sb with a
    constant-1 column prepended per head ([1 | v_h]), so the attention-V
    matmul also accumulates the softmax denominator at partition 0.
  - attention is computed transposed: S_T [tk, tq] = kT-tile.T @ qT-tile,
    both heads concurrently via PE tiling (K=64 row halves), P_T = exp(S/8)
    in one ACT op per (tk, tq-block) covering both heads, causal mask
    applied on diagonal 128x128 blocks by a DVE multiply; fully-masked
    blocks skipped and both heads column-trimmed on diagonal blocks.
  - den/numer_T: [65, tq] = [1 | v_h].T @ P_T per head (den at row 0).
  - normalize: reciprocal_approx_fast directly on the PSUM den row (base
    0), cast bf16, broadcast to 128 partitions with a K=1 PE matmul, one
    DVE multiply per head writes normalized att rows.
  - y_partial [T, D] = att.T.T @ wo_rows; PSUM evacuated by DVE (cols
    0:512) and GpSimd (cols 512:1024), DMA'd bf16 on the Sync engine;
    summed on the host.

Engine budget: ACT does exp only (its table is preloaded during the input
DMA); DVE does casts/mask/normalize + half the O-proj evac; GpSimd does the
v scatter + the other half of the O-proj evac; Sync issues all DMAs.

Scheduling: the PE has a p-state ramp (full 2.4 GHz only after ~3us of
continuous execution), so the emission order keeps the PE dense:
  - warm-up matmuls on memset scratch cover the input-DMA wait;
  - the attention i-loop is software-pipelined: S(i+1) is emitted before
    AV(i); projection/O-projection work is drained quota-wise inside the
    i-loop to fill PE slack under the ACT exp stream;
  - O-proj tiles are spread across round boundaries so the post-round-3
    tail is only normalize(3) + 4 tiles.
"""

from contextlib import ExitStack

import numpy as np
import ml_dtypes

import concourse.bacc as bacc
import concourse.mybir as mybir
import concourse.tile as tile

BF16 = ml_dtypes.bfloat16
T = 2048
D = 1024
HD = 64
N_CORES = 8
KD = D // 128          # 8 contraction chunks for projections
NT128 = T // 128       # 16
NT512 = T // 512       # 4
VS = 256               # v_sb per-tile stride: [1*64 | v_h0(64) | 1*64 | v_h1(64)]
SCALE = 1.0 / 8.0      # 1/sqrt(64)
WARM_N = 36            # p-state warm-up matmuls while input DMA is in flight

# w_all column offsets
WQ0, WK0, WV0, WO0, MSK0, ID0 = 0, D, 2 * D, 3 * D, 4 * D, 4 * D + 128
WCOLS = 4 * D + 256

F32 = mybir.dt.float32
BF = mybir.dt.bfloat16

_SENTINEL = object()


class _Work:
    """Wraps an emission generator that yields False mid-segment and True at
    segment boundaries (points where every pool accumulation it opened is
    closed, so other users of the same pools may allocate)."""

    def __init__(self, gen):
        self.gen = gen
        self.at_boundary = True
        self.done = False

    def step(self):
        r = next(self.gen, _SENTINEL)
        if r is _SENTINEL:
            self.done = True
            self.at_boundary = True
            return False
        self.at_boundary = bool(r)
        return True

    def drain_to_boundary(self):
        while not (self.at_boundary or self.done):
            self.step()

    def drain_all(self):
        while not self.done:
            self.step()


def _kernel(tc, y, xT, w_all, dbg=None):
    nc = tc.nc
    Exp = mybir.ActivationFunctionType.Exp

    with ExitStack() as ctx:
        persist = ctx.enter_context(tc.tile_pool(name="persist", bufs=1))
        ps_mm = ctx.enter_context(tc.tile_pool(name="ps_mm", bufs=2, space="PSUM"))
        ps_s = ctx.enter_context(tc.tile_pool(name="ps_s", bufs=2, space="PSUM"))
        ps_av = ctx.enter_context(tc.tile_pool(name="ps_av", bufs=1, space="PSUM"))
        pool_p = ctx.enter_context(tc.tile_pool(name="pool_p", bufs=5))
        pool_r = ctx.enter_context(tc.tile_pool(name="pool_r", bufs=2))
        pool_y = ctx.enter_context(tc.tile_pool(name="pool_y", bufs=6))

        w_sb = persist.tile([128, WCOLS], BF, tag="w")
        xT_sb = persist.tile([128, KD * T], BF, tag="xT")  # block (r,d) at (r*8+d)*512

        # ---- p-state warm-up scratch.
        warm_sb = persist.tile([128, 640], BF, tag="warm")
        nc.vector.memset(warm_sb[:], 0.5)

        # ---- input DMA: qkv weights first, then each xT quarter striped
        # across the three issue queues IN QUARTER ORDER (sync carries the
        # w tensors, so its stripe is smaller), so the DMA engines finish
        # r0 before starting r1 and projections start early.
        nc.sync.dma_start(w_sb[:, 0:3 * D], w_all[:, 0:3 * D])
        act_warm = persist.tile([1, 8], F32, tag="actwarm")
        for r in range(4):
            q0 = r * 4096
            nc.scalar.dma_start(xT_sb[:, q0:q0 + 1792], xT[:, q0:q0 + 1792])
            nc.gpsimd.dma_start(
                xT_sb[:, q0 + 1792:q0 + 3584], xT[:, q0 + 1792:q0 + 3584])
            nc.sync.dma_start(
                xT_sb[:, q0 + 3584:q0 + 4096], xT[:, q0 + 3584:q0 + 4096])
            if r == 0:
                # wo|mask|ident after the r0 stripes (needed later than wqkv)
                nc.sync.dma_start(w_sb[:, 3 * D:WCOLS], w_all[:, 3 * D:WCOLS])
                # preload the ACT exp table during the DMA wait (the first
                # activation otherwise pays ~1.3us of table load on the
                # attention critical path)
                nc.scalar.activation(
                    act_warm[:], warm_sb[0:1, 0:8], Exp, scale=SCALE)

        qT_sb = persist.tile([128, T], BF, tag="qT")   # partitions 0-63 head0, 64-127 head1
        kT_sb = persist.tile([128, T], BF, tag="kT")
        vT_sb = persist.tile([128, T], BF, tag="vT")
        v_sb = persist.tile([128, NT128 * VS], BF, tag="v")
        v_cols = v_sb[:].rearrange("p (t s) -> p t s", s=VS)
        nc.vector.memset(v_cols[:, :, 0:64], 1.0)      # den-broadcast columns
        nc.vector.memset(v_cols[:, :, 128:192], 1.0)
        att_sb = persist.tile([128, T], BF, tag="att")  # normalized numer_T

        wq_sb = w_sb[:, WQ0:WQ0 + D]
        wk_sb = w_sb[:, WK0:WK0 + D]
        wv_sb = w_sb[:, WV0:WV0 + D]
        wo_sb = w_sb[:, WO0:WO0 + D]
        mask_sb = w_sb[:, MSK0:MSK0 + 128]
        id_sb = w_sb[:, ID0:ID0 + 128]

        def proj_gen(r):
            """QKV projections for column-block r: 24 N=512 matmuls + 3 casts
            + 4 transposes (+ GpSimd scatters). Yields True when the open
            ps_mm segment has been closed."""
            for w_part, dst in ((wq_sb, qT_sb), (wk_sb, kT_sb), (wv_sb, vT_sb)):
                ps = ps_mm.tile([128, 512], F32, tag="mm")
                for d in range(KD):
                    nc.tensor.matmul(
                        ps[:],
                        lhsT=w_part[:, d * 128:(d + 1) * 128],
                        rhs=xT_sb[:, (r * KD + d) * 512:(r * KD + d + 1) * 512],
                        start=(d == 0), stop=(d == KD - 1),
                    )
                    yield False
                nc.vector.tensor_copy(dst[:, r * 512:(r + 1) * 512], ps[:])
                yield True
            for t in range(4 * r, 4 * r + 4):
                ps_t = ps_mm.tile([128, 128], BF, tag="mm")
                nc.tensor.transpose(
                    ps_t[:], vT_sb[:, t * 128:(t + 1) * 128], id_sb)
                yield False
                dst = v_cols[:, t, :].rearrange(
                    "p (g q) -> p g q", q=128)[:, :, 64:128]
                src = ps_t[:].rearrange("p (g q) -> p g q", q=64)
                nc.vector.tensor_copy(dst, src)
                yield True

        def oproj_gen(tiles, dve_only=False, act_evac=False):
            """O-projection rows for the given T-chunk indices; evacuation
            split DVE (cols 0:512) / ACT (cols 512:1024) so neither engine
            builds a backlog that would delay the round-boundary normalize.
            dve_only keeps evacs off ACT when the round is exp-paced;
            act_evac puts both on ACT (tail: DVE runs normalize(3))."""
            for t in tiles:
                y_sb = pool_y.tile([128, 1024], BF, tag="y")
                ps0 = ps_mm.tile([128, 512], F32, tag="mm")
                nc.tensor.matmul(
                    ps0[:], lhsT=att_sb[:, t * 128:(t + 1) * 128],
                    rhs=wo_sb[:, 0:512], start=True, stop=True,
                )
                yield False
                ps1 = ps_mm.tile([128, 512], F32, tag="mm")
                nc.tensor.matmul(
                    ps1[:], lhsT=att_sb[:, t * 128:(t + 1) * 128],
                    rhs=wo_sb[:, 512:1024], start=True, stop=True,
                )
                yield False
                if act_evac:
                    nc.scalar.copy(y_sb[:, 0:512], ps0[:])
                else:
                    nc.vector.tensor_copy(y_sb[:, 0:512], ps0[:])
                yield True
                if dve_only:
                    nc.vector.tensor_copy(y_sb[:, 512:1024], ps1[:])
                else:
                    nc.scalar.copy(y_sb[:, 512:1024], ps1[:])
                yield True
                nc.sync.dma_start(y[t * 128:(t + 1) * 128, :], y_sb[:])
                yield True

        def oproj_tail_gen(tiles):
            """Tail O-projection: the S-pipeline PSUM banks are free, so use
            [128,1024] ps_s tiles (both matmuls in flight per tile, no
            ps_mm round-trip). The first half of the tiles evacuates on ACT
            alone (DVE is still running normalize(3)'s second chunk); the
            rest split DVE/ACT. Each half DMAs out as soon as it lands."""
            for idx, t in enumerate(tiles):
                y_sb = pool_y.tile([128, 1024], BF, tag="y")
                ps = ps_s.tile([128, 1024], F32, tag="s")
                nc.tensor.matmul(
                    ps[:, 0:512], lhsT=att_sb[:, t * 128:(t + 1) * 128],
                    rhs=wo_sb[:, 0:512], start=True, stop=True,
                )
                yield False
                nc.tensor.matmul(
                    ps[:, 512:1024], lhsT=att_sb[:, t * 128:(t + 1) * 128],
                    rhs=wo_sb[:, 512:1024], start=True, stop=True,
                )
                yield False
                if idx < len(tiles) // 2:
                    nc.scalar.copy(y_sb[:, 0:512], ps[:, 0:512])
                else:
                    nc.vector.tensor_copy(y_sb[:, 0:512], ps[:, 0:512])
                yield True
                nc.scalar.copy(y_sb[:, 512:1024], ps[:, 512:1024])
                yield True
                eng = nc.gpsimd if t % 2 else nc.sync
                eng.dma_start(y[t * 128:(t + 1) * 128, :], y_sb[:])
                yield True

        def S_pair(j, i):
            """S_T block matmuls for (tq round j, tk block i): both heads
            concurrently on disjoint PE row-halves (K=64 tile positions)."""
            m = i - 4 * j
            off = 128 * m if m > 0 else 0
            s_pair = ps_s.tile([128, 1024], F32, tag="s")
            nc.tensor.matmul(
                s_pair[:, off:512],
                lhsT=kT_sb[0:64, i * 128:(i + 1) * 128],
                rhs=qT_sb[0:64, j * 512 + off:(j + 1) * 512],
                start=True, stop=True, tile_position=(0, 0),
            )
            nc.tensor.matmul(
                s_pair[:, 512:1024 - off],
                lhsT=kT_sb[64:128, i * 128:(i + 1) * 128],
                rhs=qT_sb[64:128, j * 512 + off:(j + 1) * 512],
                start=True, stop=True, tile_position=(64, 0),
            )
            return s_pair

        def attn_round(j, work, total, reserve, s0=None):
            """Causal attention for tq block j, software-pipelined; spreads
            ~(total - reserve) interleave items evenly over the i-loop so the
            PE stays ahead of ACT on every iteration. s0 is the pre-emitted
            S-pair for i=0 (emitted before the previous round boundary's
            PSUM-gated filler, so the PE never goes idle at the boundary)."""
            n_i = 4 * j + 4
            remaining = max(0, total - reserve)
            avden = ps_av.tile([128, 1024], F32, tag="avden")
            s_tiles = [None] * n_i

            s_tiles[0] = s0 if s0 is not None else S_pair(j, 0)
            for i in range(n_i):
                if i + 1 < n_i:
                    s_tiles[i + 1] = S_pair(j, i + 1)
                m = i - 4 * j
                off = 128 * m if m > 0 else 0
                ncol = 512 - off
                first, last = (i == 0), (i == n_i - 1)
                s_pair = s_tiles[i]
                s_tiles[i] = None
                p_sb = pool_p.tile([128, 1024], BF, tag="p")
                nc.scalar.activation(
                    p_sb[:, off:512 + ncol], s_pair[:, off:512 + ncol], Exp,
                    scale=SCALE,
                )
                if m >= 0:  # causal mask on the 128x128 diagonal sub-blocks
                    nc.vector.tensor_mul(
                        p_sb[:, off:off + 128], p_sb[:, off:off + 128], mask_sb,
                    )
                    nc.vector.tensor_mul(
                        p_sb[:, 512:640], p_sb[:, 512:640], mask_sb,
                    )
                # interleave filler BEFORE the AV pair: the PE chews on it
                # while ACT finishes exp(i) (and the pool the mask), instead
                # of idling in-order behind the AV's data dependency.
                k = -(-remaining // (n_i - i))  # ceil: spread evenly
                remaining -= k
                for _ in range(k):
                    if not work.step():
                        break
                nc.tensor.matmul(
                    avden[:, off:512],
                    lhsT=v_sb[:, VS * i: VS * i + 128],
                    rhs=p_sb[:, off:512],
                    start=first, stop=last,
                )
                nc.tensor.matmul(
                    avden[:, 512 + off:1024],
                    lhsT=v_sb[:, VS * i + 128: VS * i + 256],
                    rhs=p_sb[:, 512:512 + ncol],
                    start=first, stop=last,
                )
            return avden

        def normalize(j, avden, split=1):
            """The AV matmul already broadcast the denominator to partitions
            0-63 (64 ones columns in the v lhsT); recip it straight from
            PSUM (base 0) and one DVE mul per head writes normalized att.
            split>1 processes tq column chunks separately so downstream
            O-proj tiles unlock as soon as their chunk is normalized."""
            w = 512 // split
            recs = {}
            for c in range(split):
                for h in range(2):
                    rec = pool_r.tile([64, w], F32, tag=f"rec{c}{h}")
                    nc.vector.reciprocal_approx_fast(
                        rec[:], avden[0:64, h * 512 + c * w:h * 512 + (c + 1) * w])
                    recs[c, h] = rec
            for c in range(split):
                for h in range(2):
                    nc.vector.tensor_mul(
                        att_sb[h * 64:(h + 1) * 64,
                               j * 512 + c * w:j * 512 + (c + 1) * w],
                        avden[64:128, h * 512 + c * w:h * 512 + (c + 1) * w],
                        recs[c, h][:],
                    )

        # warm-up stream, then round 0 projections (PE hot when xT lands)
        warm_ps = ps_s.tile([128, 1024], F32, tag="s")
        for _ in range(WARM_N):
            nc.tensor.matmul(
                warm_ps[:, 0:512], lhsT=warm_sb[:, 0:128], rhs=warm_sb[:, 128:640],
                start=True, stop=True,
            )
        for _ in proj_gen(0):
            pass

        # r=0: attn(0) ⟂ proj(1); normalize(0); proj(1) rest; oproj t 0-1
        # r=1: attn(1) ⟂ proj(2); normalize(1); proj(2) rest; oproj t 2-3
        # r=2: attn(2) ⟂ proj(3); normalize(2); proj(3) rest; oproj t 4-5
        # r=3: attn(3) ⟂ oproj t 6-7 (small: attn3 is ACT-paced);
        #      normalize(3) with oproj t 8-11 matmuls feeding the PE while
        #      the DVE runs the recip/mul chain; then the ps_s-based tail.
        work = _Work(proj_gen(1))
        s0 = None
        for r in range(NT512):
            if r < 3:
                avden = attn_round(r, work, 35, 6, s0=s0)
            else:
                avden = attn_round(r, work, 35, 4, s0=s0)
            # pre-emit the next round's first S-pair so the PE has
            # dependency-free work queued ahead of the PSUM-slot-gated
            # boundary fillers (in-order head-of-line blocking otherwise
            # idles the PE and drops its p-state clock)
            s0 = S_pair(r + 1, 0) if r + 1 < NT512 else None
            # normalize only touches pool_r/avden/att, so emit it before the
            # leftover work: its DVE ops jump ahead of the remaining evacs
            normalize(r, avden, split=(2 if r == 3 else 1))
            work.drain_all()
            if r == 0:
                _Work(oproj_gen([0, 1])).drain_all()
                work = _Work(proj_gen(2))
            elif r == 1:
                work = _Work(proj_gen(3))
            elif r == 2:
                work = _Work(
                    oproj_gen([2, 3, 4, 5, 6, 7, 8], dve_only=True))
            else:
                _Work(oproj_gen([9, 10, 11], act_evac=True)).drain_all()
                _Work(oproj_tail_gen([12, 13, 14, 15])).drain_all()

        if dbg is not None:
            for name, sb in (("qT", qT_sb), ("kT", kT_sb), ("vT", vT_sb),
                             ("att", att_sb)):
                nc.sync.dma_start(dbg[name][:], sb[:])
            nc.sync.dma_start(dbg["v"][:], v_sb[:])
            nc.sync.dma_start(dbg["xT"][:], xT_sb[:])
            nc.sync.dma_start(dbg["w"][:], w_sb[:])


def _build_program(debug_dumps=False):
    nc = bacc.Bacc("TRN2", debug=False, num_devices=N_CORES)
    xT = nc.dram_tensor("xT", [128, KD * T], BF, kind="ExternalInput").ap()
    w_all = nc.dram_tensor("w_all", [128, WCOLS], BF, kind="ExternalInput").ap()
    y = nc.dram_tensor("y", [T, D], BF, kind="ExternalOutput").ap()
    dbg = None
    if debug_dumps:
        dbg = {
            name: nc.dram_tensor(f"dbg_{name}", [128, T], BF, kind="ExternalOutput").ap()
            for name in ("qT", "kT", "vT", "att")
        }
        dbg["v"] = nc.dram_tensor("dbg_v", [128, NT128 * VS], BF, kind="ExternalOutput").ap()
        dbg["xT"] = nc.dram_tensor("dbg_xT", [128, KD * T], BF, kind="ExternalOutput").ap()
        dbg["w"] = nc.dram_tensor("dbg_w", [128, WCOLS], BF, kind="ExternalOutput").ap()

    with tile.TileContext(nc) as tc:
        _kernel(tc, y, xT, w_all, dbg=dbg)
    nc.compile()
    return nc


_NC = None


def _get_program():
    global _NC
    if _NC is None:
        _NC = _build_program()
    return _NC


def _rearrange_w(w_cols):
    """[1024, 128] f32 slice of W_qkv -> [128, 1024] bf16 with d-chunk d at
    cols [d*128, (d+1)*128): out[p, d*128 + m] = w_cols[d*128 + p, m]."""
    return np.ascontiguousarray(
        w_cols.reshape(KD, 128, 128).transpose(1, 0, 2).reshape(128, KD * 128)
    ).astype(BF16)


def make_in_maps(x, W_qkv, W_o):
    x2 = np.asarray(x, dtype=np.float32).reshape(T, D)
    W_qkv = np.asarray(W_qkv, dtype=np.float32)
    W_o = np.asarray(W_o, dtype=np.float32)

    # xT blocked [p, r, d, c]: col (r*8+d)*512 + c holds x[r*512+c, d*128+p]
    xT_bf = np.ascontiguousarray(
        x2.T.reshape(KD, 128, NT512, 512).transpose(1, 2, 0, 3).reshape(128, KD * T)
    ).astype(BF16)
    mask = np.triu(np.ones((128, 128), dtype=np.float32)).astype(BF16)
    ident = np.eye(128, dtype=np.float32).astype(BF16)

    in_maps = []
    for c in range(N_CORES):
        cs = slice(2 * c * HD, 2 * c * HD + 128)
        w_all = np.concatenate([
            _rearrange_w(W_qkv[:, 0 * D:1 * D][:, cs]),
            _rearrange_w(W_qkv[:, 1 * D:2 * D][:, cs]),
            _rearrange_w(W_qkv[:, 2 * D:3 * D][:, cs]),
            np.ascontiguousarray(W_o[c * 128:(c + 1) * 128, :]).astype(BF16),
            mask,
            ident,
        ], axis=1)
        in_maps.append({"xT": xT_bf, "w_all": w_all})
    return in_maps


def combine_outputs(results):
    y_full = np.zeros((T, D), dtype=np.float32)
    for c in range(N_CORES):
        y_full += results[c]["y"].astype(np.float32)
    return y_full.reshape(1, T, D)


def kernel(x, W_qkv, W_o):
    from concourse.bass_utils import run_bass_kernel_spmd

    nc = _get_program()
    in_maps = make_in_maps(x, W_qkv, W_o)
    res = run_bass_kernel_spmd(nc, in_maps, core_ids=list(range(N_CORES)))
    return combine_outputs(res.results)


# revision 66
# speedup vs baseline: 1.2082x; 1.0230x over previous
"""Trainium2 Bass kernel for causal multi-head attention with QKV/O projections.

Problem: x [1, 2048, 1024] f32, W_qkv [1024, 3072] (q|k|v blocks), W_o
[1024, 1024], H=16 heads, head_dim=64, dense causal attention,
y = softmax(q k^T / 8, causal) v, out = y @ W_o.

Sharding: head-parallel over 8 NeuronCores (2 heads per core). Each core
computes q/k/v projections for its 2 heads, causal attention, and a partial
O-projection (its 128 attention-output columns against its 128 rows of W_o).
The host sums the 8 partial outputs.

On-core dataflow (bf16 into the PE, f32 accumulation in PSUM):
  - xT [128, (r d c)] arrives pre-transposed and pre-blocked from the host
    (column block r, contraction chunk d), so the r=0 quarter lands first
    and projections start before the rest of x arrives.
  - all weights + mask + identity arrive in ONE dram tensor w_all
    [128, 4352] = wq|wk|wv|wo|mask|ident (one DMA issue instead of six).
  - qT/kT/vT [128, T] = W.T @ xT (2 heads stacked on partitions); v is then
    materialized in [tk, hd] layout via PE transposes of vT 128x128 blocks
    (bf16 transpose datapath), evacuated by GpSimd into v_sb with a
    constant-1 column prepended per head ([1 | v_h]), so the attention-V
    matmul also accumulates the softmax denominator at partition 0.
  - attention is computed transposed: S_T [tk, tq] = kT-tile.T @ qT-tile,
    both heads concurrently via PE tiling (K=64 row halves), P_T = exp(S/8)
    in one ACT op per (tk, tq-block) covering both heads, causal mask
    applied on diagonal 128x128 blocks by a DVE multiply; fully-masked
    blocks skipped and both heads column-trimmed on diagonal blocks.
  - den/numer_T: [65, tq] = [1 | v_h].T @ P_T per head (den at row 0).
  - normalize: reciprocal_approx_fast directly on the PSUM den row (base
    0), cast bf16, broadcast to 128 partitions with a K=1 PE matmul, one
    DVE multiply per head writes normalized att rows.
  - y_partial [T, D] = att.T.T @ wo_rows; PSUM evacuated by DVE (cols
    0:512) and GpSimd (cols 512:1024), DMA'd bf16 on the Sync engine;
    summed on the host.

Engine budget: ACT does exp only (its table is preloaded during the input
DMA); DVE does casts/mask/normalize + half the O-proj evac; GpSimd does the
v scatter + the other half of the O-proj evac; Sync issues all DMAs.

Scheduling: the PE has a p-state ramp (full 2.4 GHz only after ~3us of
continuous execution), so the emission order keeps the PE dense:
  - warm-up matmuls on memset scratch cover the input-DMA wait;
  - the attention i-loop is software-pipelined: S(i+1) is emitted before
    AV(i); projection/O-projection work is drained quota-wise inside the
    i-loop to fill PE slack under the ACT exp stream;
  - O-proj tiles are spread across round boundaries so the post-round-3
    tail is only normalize(3) + 4 tiles.
"""

from contextlib import ExitStack

import numpy as np
import ml_dtypes

import concourse.bacc as bacc
import concourse.mybir as mybir
import concourse.tile as tile

BF16 = ml_dtypes.bfloat16
T = 2048
D = 1024
HD = 64
N_CORES = 8
KD = D // 128          # 8 contraction chunks for projections
NT128 = T // 128       # 16
NT512 = T // 512       # 4
VS = 256               # v_sb per-tile stride: [1*64 | v_h0(64) | 1*64 | v_h1(64)]
SCALE = 1.0 / 8.0      # 1/sqrt(64)
WARM_N = 36            # p-state warm-up matmuls while input DMA is in flight

# w_all column offsets
WQ0, WK0, WV0, WO0, MSK0, ID0 = 0, D, 2 * D, 3 * D, 4 * D, 4 * D + 128
WCOLS = 4 * D + 256

F32 = mybir.dt.float32
BF = mybir.dt.bfloat16

_SENTINEL = object()


class _Work:
    """Wraps an emission generator that yields False mid-segment and True at
    segment boundaries (points where every pool accumulation it opened is
    closed, so other users of the same pools may allocate)."""

    def __init__(self, gen):
        self.gen = gen
        self.at_boundary = True
        self.done = False

    def step(self):
        r = next(self.gen, _SENTINEL)
        if r is _SENTINEL:
            self.done = True
            self.at_boundary = True
            return False
        self.at_boundary = bool(r)
        return True

    def drain_to_boundary(self):
        while not (self.at_boundary or self.done):
            self.step()

    def drain_all(self):
        while not self.done:
            self.step()


def _kernel(tc, y, xT, w_all, dbg=None):
    nc = tc.nc
    Exp = mybir.ActivationFunctionType.Exp

    with ExitStack() as ctx:
        persist = ctx.enter_context(tc.tile_pool(name="persist", bufs=1))
        ps_mm = ctx.enter_context(tc.tile_pool(name="ps_mm", bufs=2, space="PSUM"))
        ps_s = ctx.enter_context(tc.tile_pool(name="ps_s", bufs=2, space="PSUM"))
        ps_av = ctx.enter_context(tc.tile_pool(name="ps_av", bufs=1, space="PSUM"))
        pool_p = ctx.enter_context(tc.tile_pool(name="pool_p", bufs=5))
        pool_r = ctx.enter_context(tc.tile_pool(name="pool_r", bufs=2))
        pool_y = ctx.enter_context(tc.tile_pool(name="pool_y", bufs=6))

        w_sb = persist.tile([128, WCOLS], BF, tag="w")
        xT_sb = persist.tile([128, KD * T], BF, tag="xT")  # block (r,d) at (r*8+d)*512

        # ---- p-state warm-up scratch.
        warm_sb = persist.tile([128, 640], BF, tag="warm")
        nc.vector.memset(warm_sb[:], 0.5)

        # ---- input DMA: qkv weights first, then each xT quarter striped
        # across the three issue queues IN QUARTER ORDER (sync carries the
        # w tensors, so its stripe is smaller), so the DMA engines finish
        # r0 before starting r1 and projections start early.
        nc.sync.dma_start(w_sb[:, 0:3 * D], w_all[:, 0:3 * D])
        act_warm = persist.tile([1, 8], F32, tag="actwarm")
        for r in range(4):
            q0 = r * 4096
            nc.scalar.dma_start(xT_sb[:, q0:q0 + 1792], xT[:, q0:q0 + 1792])
            nc.gpsimd.dma_start(
                xT_sb[:, q0 + 1792:q0 + 3584], xT[:, q0 + 1792:q0 + 3584])
            nc.sync.dma_start(
                xT_sb[:, q0 + 3584:q0 + 4096], xT[:, q0 + 3584:q0 + 4096])
            if r == 0:
                # wo|mask|ident after the r0 stripes (needed later than wqkv)
                nc.sync.dma_start(w_sb[:, 3 * D:WCOLS], w_all[:, 3 * D:WCOLS])
                # preload the ACT exp table during the DMA wait (the first
                # activation otherwise pays ~1.3us of table load on the
                # attention critical path)
                nc.scalar.activation(
                    act_warm[:], warm_sb[0:1, 0:8], Exp, scale=SCALE)

        qT_sb = persist.tile([128, T], BF, tag="qT")   # partitions 0-63 head0, 64-127 head1
        kT_sb = persist.tile([128, T], BF, tag="kT")
        vT_sb = persist.tile([128, T], BF, tag="vT")
        v_sb = persist.tile([128, NT128 * VS], BF, tag="v")
        v_cols = v_sb[:].rearrange("p (t s) -> p t s", s=VS)
        nc.vector.memset(v_cols[:, :, 0:64], 1.0)      # den-broadcast columns
        nc.vector.memset(v_cols[:, :, 128:192], 1.0)
        att_sb = persist.tile([128, T], BF, tag="att")  # normalized numer_T

        wq_sb = w_sb[:, WQ0:WQ0 + D]
        wk_sb = w_sb[:, WK0:WK0 + D]
        wv_sb = w_sb[:, WV0:WV0 + D]
        wo_sb = w_sb[:, WO0:WO0 + D]
        mask_sb = w_sb[:, MSK0:MSK0 + 128]
        id_sb = w_sb[:, ID0:ID0 + 128]

        def proj_gen(r):
            """QKV projections for column-block r: 24 N=512 matmuls + 3 casts
            + 4 transposes (+ GpSimd scatters). Yields True when the open
            ps_mm segment has been closed."""
            for w_part, dst in ((wq_sb, qT_sb), (wk_sb, kT_sb), (wv_sb, vT_sb)):
                ps = ps_mm.tile([128, 512], F32, tag="mm")
                for d in range(KD):
                    nc.tensor.matmul(
                        ps[:],
                        lhsT=w_part[:, d * 128:(d + 1) * 128],
                        rhs=xT_sb[:, (r * KD + d) * 512:(r * KD + d + 1) * 512],
                        start=(d == 0), stop=(d == KD - 1),
                    )
                    yield False
                nc.vector.tensor_copy(dst[:, r * 512:(r + 1) * 512], ps[:])
                yield True
            for t in range(4 * r, 4 * r + 4):
                ps_t = ps_mm.tile([128, 128], BF, tag="mm")
                nc.tensor.transpose(
                    ps_t[:], vT_sb[:, t * 128:(t + 1) * 128], id_sb)
                yield False
                dst = v_cols[:, t, :].rearrange(
                    "p (g q) -> p g q", q=128)[:, :, 64:128]
                src = ps_t[:].rearrange("p (g q) -> p g q", q=64)
                nc.vector.tensor_copy(dst, src)
                yield True

        def oproj_gen(tiles, dve_only=False, act_evac=False):
            """O-projection rows for the given T-chunk indices; evacuation
            split DVE (cols 0:512) / ACT (cols 512:1024) so neither engine
            builds a backlog that would delay the round-boundary normalize.
            dve_only keeps evacs off ACT when the round is exp-paced;
            act_evac puts both on ACT (tail: DVE runs normalize(3))."""
            for t in tiles:
                y_sb = pool_y.tile([128, 1024], BF, tag="y")
                ps0 = ps_mm.tile([128, 512], F32, tag="mm")
                nc.tensor.matmul(
                    ps0[:], lhsT=att_sb[:, t * 128:(t + 1) * 128],
                    rhs=wo_sb[:, 0:512], start=True, stop=True,
                )
                yield False
                ps1 = ps_mm.tile([128, 512], F32, tag="mm")
                nc.tensor.matmul(
                    ps1[:], lhsT=att_sb[:, t * 128:(t + 1) * 128],
                    rhs=wo_sb[:, 512:1024], start=True, stop=True,
                )
                yield False
                if act_evac:
                    nc.scalar.copy(y_sb[:, 0:512], ps0[:])
                else:
                    nc.vector.tensor_copy(y_sb[:, 0:512], ps0[:])
                yield True
                if dve_only:
                    nc.vector.tensor_copy(y_sb[:, 512:1024], ps1[:])
                else:
                    nc.scalar.copy(y_sb[:, 512:1024], ps1[:])
                yield True
                nc.sync.dma_start(y[t * 128:(t + 1) * 128, :], y_sb[:])
                yield True

        def oproj_tail_gen(tiles):
            """Tail O-projection: the S-pipeline PSUM banks are free, so use
            [128,1024] ps_s tiles (both matmuls in flight per tile, no
            ps_mm round-trip). The first half of the tiles evacuates on ACT
            alone (DVE is still running normalize(3)'s second chunk); the
            rest split DVE/ACT. Each half DMAs out as soon as it lands."""
            for idx, t in enumerate(tiles):
                y_sb = pool_y.tile([128, 1024], BF, tag="y")
                ps = ps_s.tile([128, 1024], F32, tag="s")
                nc.tensor.matmul(
                    ps[:, 0:512], lhsT=att_sb[:, t * 128:(t + 1) * 128],
                    rhs=wo_sb[:, 0:512], start=True, stop=True,
                )
                yield False
                nc.tensor.matmul(
                    ps[:, 512:1024], lhsT=att_sb[:, t * 128:(t + 1) * 128],
                    rhs=wo_sb[:, 512:1024], start=True, stop=True,
                )
                yield False
                if idx == 0:
                    nc.scalar.copy(y_sb[:, 0:512], ps[:, 0:512])
                else:
                    nc.vector.tensor_copy(y_sb[:, 0:512], ps[:, 0:512])
                yield True
                nc.scalar.copy(y_sb[:, 512:1024], ps[:, 512:1024])
                yield True
                eng = nc.gpsimd if t % 2 else nc.sync
                eng.dma_start(y[t * 128:(t + 1) * 128, :], y_sb[:])
                yield True

        def S_pair(j, i):
            """S_T block matmuls for (tq round j, tk block i): both heads
            concurrently on disjoint PE row-halves (K=64 tile positions)."""
            m = i - 4 * j
            off = 128 * m if m > 0 else 0
            s_pair = ps_s.tile([128, 1024], F32, tag="s")
            nc.tensor.matmul(
                s_pair[:, off:512],
                lhsT=kT_sb[0:64, i * 128:(i + 1) * 128],
                rhs=qT_sb[0:64, j * 512 + off:(j + 1) * 512],
                start=True, stop=True, tile_position=(0, 0),
            )
            nc.tensor.matmul(
                s_pair[:, 512:1024 - off],
                lhsT=kT_sb[64:128, i * 128:(i + 1) * 128],
                rhs=qT_sb[64:128, j * 512 + off:(j + 1) * 512],
                start=True, stop=True, tile_position=(64, 0),
            )
            return s_pair

        def attn_round(j, work, total, reserve, s0=None):
            """Causal attention for tq block j, software-pipelined; spreads
            ~(total - reserve) interleave items evenly over the i-loop so the
            PE stays ahead of ACT on every iteration. s0 is the pre-emitted
            S-pair for i=0 (emitted before the previous round boundary's
            PSUM-gated filler, so the PE never goes idle at the boundary)."""
            n_i = 4 * j + 4
            remaining = max(0, total - reserve)
            avden = ps_av.tile([128, 1024], F32, tag="avden")
            s_tiles = [None] * n_i

            s_tiles[0] = s0 if s0 is not None else S_pair(j, 0)
            for i in range(n_i):
                if i + 1 < n_i:
                    s_tiles[i + 1] = S_pair(j, i + 1)
                m = i - 4 * j
                off = 128 * m if m > 0 else 0
                ncol = 512 - off
                first, last = (i == 0), (i == n_i - 1)
                s_pair = s_tiles[i]
                s_tiles[i] = None
                p_sb = pool_p.tile([128, 1024], BF, tag="p")
                nc.scalar.activation(
                    p_sb[:, off:512 + ncol], s_pair[:, off:512 + ncol], Exp,
                    scale=SCALE,
                )
                if m >= 0:  # causal mask on the 128x128 diagonal sub-blocks
                    nc.vector.tensor_mul(
                        p_sb[:, off:off + 128], p_sb[:, off:off + 128], mask_sb,
                    )
                    nc.vector.tensor_mul(
                        p_sb[:, 512:640], p_sb[:, 512:640], mask_sb,
                    )
                # interleave filler BEFORE the AV pair: the PE chews on it
                # while ACT finishes exp(i) (and the pool the mask), instead
                # of idling in-order behind the AV's data dependency.
                k = -(-remaining // (n_i - i))  # ceil: spread evenly
                remaining -= k
                for _ in range(k):
                    if not work.step():
                        break
                nc.tensor.matmul(
                    avden[:, off:512],
                    lhsT=v_sb[:, VS * i: VS * i + 128],
                    rhs=p_sb[:, off:512],
                    start=first, stop=last,
                )
                nc.tensor.matmul(
                    avden[:, 512 + off:1024],
                    lhsT=v_sb[:, VS * i + 128: VS * i + 256],
                    rhs=p_sb[:, 512:512 + ncol],
                    start=first, stop=last,
                )
            return avden

        def normalize(j, avden, split=1):
            """The AV matmul already broadcast the denominator to partitions
            0-63 (64 ones columns in the v lhsT); recip it straight from
            PSUM (base 0) and one DVE mul per head writes normalized att.
            split>1 processes tq column chunks separately so downstream
            O-proj tiles unlock as soon as their chunk is normalized."""
            w = 512 // split
            for c in range(split):
                recs = []
                for h in range(2):
                    rec = pool_r.tile([64, w], F32, tag=f"rec{c}")
                    nc.vector.reciprocal_approx_fast(
                        rec[:], avden[0:64, h * 512 + c * w:h * 512 + (c + 1) * w])
                    recs.append(rec)
                for h in range(2):
                    nc.vector.tensor_mul(
                        att_sb[h * 64:(h + 1) * 64,
                               j * 512 + c * w:j * 512 + (c + 1) * w],
                        avden[64:128, h * 512 + c * w:h * 512 + (c + 1) * w],
                        recs[h][:],
                    )

        # warm-up stream, then round 0 projections (PE hot when xT lands)
        warm_ps = ps_s.tile([128, 1024], F32, tag="s")
        for _ in range(WARM_N):
            nc.tensor.matmul(
                warm_ps[:, 0:512], lhsT=warm_sb[:, 0:128], rhs=warm_sb[:, 128:640],
                start=True, stop=True,
            )
        for _ in proj_gen(0):
            pass

        # r=0: attn(0) ⟂ proj(1); normalize(0); proj(1) rest; oproj t 0-1
        # r=1: attn(1) ⟂ proj(2); normalize(1); proj(2) rest; oproj t 2-3
        # r=2: attn(2) ⟂ proj(3); normalize(2); proj(3) rest; oproj t 4-5
        # r=3: attn(3) ⟂ oproj t 6-7 (small: attn3 is ACT-paced);
        #      normalize(3) with oproj t 8-11 matmuls feeding the PE while
        #      the DVE runs the recip/mul chain; then the ps_s-based tail.
        work = _Work(proj_gen(1))
        s0 = None
        for r in range(NT512):
            if r < 3:
                avden = attn_round(r, work, 35, 6, s0=s0)
            else:
                avden = attn_round(r, work, 35, 4, s0=s0)
            # pre-emit the next round's first S-pair so the PE has
            # dependency-free work queued ahead of the PSUM-slot-gated
            # boundary fillers (in-order head-of-line blocking otherwise
            # idles the PE and drops its p-state clock)
            s0 = S_pair(r + 1, 0) if r + 1 < NT512 else None
            # normalize only touches pool_r/avden/att, so emit it before the
            # leftover work: its DVE ops jump ahead of the remaining evacs
            normalize(r, avden, split=(2 if r == 3 else 1))
            work.drain_all()
            if r == 0:
                _Work(oproj_gen([0, 1])).drain_all()
                work = _Work(proj_gen(2))
            elif r == 1:
                work = _Work(proj_gen(3))
            elif r == 2:
                work = _Work(
                    oproj_gen([2, 3, 4, 5, 6, 7, 8], dve_only=True))
            else:
                _Work(oproj_gen([9, 10, 11], act_evac=True)).drain_all()
                _Work(oproj_tail_gen([12, 13, 14, 15])).drain_all()

        if dbg is not None:
            for name, sb in (("qT", qT_sb), ("kT", kT_sb), ("vT", vT_sb),
                             ("att", att_sb)):
                nc.sync.dma_start(dbg[name][:], sb[:])
            nc.sync.dma_start(dbg["v"][:], v_sb[:])
            nc.sync.dma_start(dbg["xT"][:], xT_sb[:])
            nc.sync.dma_start(dbg["w"][:], w_sb[:])


def _build_program(debug_dumps=False):
    nc = bacc.Bacc("TRN2", debug=False, num_devices=N_CORES)
    xT = nc.dram_tensor("xT", [128, KD * T], BF, kind="ExternalInput").ap()
    w_all = nc.dram_tensor("w_all", [128, WCOLS], BF, kind="ExternalInput").ap()
    y = nc.dram_tensor("y", [T, D], BF, kind="ExternalOutput").ap()
    dbg = None
    if debug_dumps:
        dbg = {
            name: nc.dram_tensor(f"dbg_{name}", [128, T], BF, kind="ExternalOutput").ap()
            for name in ("qT", "kT", "vT", "att")
        }
        dbg["v"] = nc.dram_tensor("dbg_v", [128, NT128 * VS], BF, kind="ExternalOutput").ap()
        dbg["xT"] = nc.dram_tensor("dbg_xT", [128, KD * T], BF, kind="ExternalOutput").ap()
        dbg["w"] = nc.dram_tensor("dbg_w", [128, WCOLS], BF, kind="ExternalOutput").ap()

    with tile.TileContext(nc) as tc:
        _kernel(tc, y, xT, w_all, dbg=dbg)
    nc.compile()
    return nc


_NC = None


def _get_program():
    global _NC
    if _NC is None:
        _NC = _build_program()
    return _NC


def _rearrange_w(w_cols):
    """[1024, 128] f32 slice of W_qkv -> [128, 1024] bf16 with d-chunk d at
    cols [d*128, (d+1)*128): out[p, d*128 + m] = w_cols[d*128 + p, m]."""
    return np.ascontiguousarray(
        w_cols.reshape(KD, 128, 128).transpose(1, 0, 2).reshape(128, KD * 128)
    ).astype(BF16)


def make_in_maps(x, W_qkv, W_o):
    x2 = np.asarray(x, dtype=np.float32).reshape(T, D)
    W_qkv = np.asarray(W_qkv, dtype=np.float32)
    W_o = np.asarray(W_o, dtype=np.float32)

    # xT blocked [p, r, d, c]: col (r*8+d)*512 + c holds x[r*512+c, d*128+p]
    xT_bf = np.ascontiguousarray(
        x2.T.reshape(KD, 128, NT512, 512).transpose(1, 2, 0, 3).reshape(128, KD * T)
    ).astype(BF16)
    mask = np.triu(np.ones((128, 128), dtype=np.float32)).astype(BF16)
    ident = np.eye(128, dtype=np.float32).astype(BF16)

    in_maps = []
    for c in range(N_CORES):
        cs = slice(2 * c * HD, 2 * c * HD + 128)
        w_all = np.concatenate([
            _rearrange_w(W_qkv[:, 0 * D:1 * D][:, cs]),
            _rearrange_w(W_qkv[:, 1 * D:2 * D][:, cs]),
            _rearrange_w(W_qkv[:, 2 * D:3 * D][:, cs]),
            np.ascontiguousarray(W_o[c * 128:(c + 1) * 128, :]).astype(BF16),
            mask,
            ident,
        ], axis=1)
        in_maps.append({"xT": xT_bf, "w_all": w_all})
    return in_maps


def combine_outputs(results):
    y_full = np.zeros((T, D), dtype=np.float32)
    for c in range(N_CORES):
        y_full += results[c]["y"].astype(np.float32)
    return y_full.reshape(1, T, D)


def kernel(x, W_qkv, W_o):
    from concourse.bass_utils import run_bass_kernel_spmd

    nc = _get_program()
    in_maps = make_in_maps(x, W_qkv, W_o)
    res = run_bass_kernel_spmd(nc, in_maps, core_ids=list(range(N_CORES)))
    return combine_outputs(res.results)


# revision 69
# speedup vs baseline: 1.2318x; 1.0195x over previous
"""Trainium2 Bass kernel for causal multi-head attention with QKV/O projections.

Problem: x [1, 2048, 1024] f32, W_qkv [1024, 3072] (q|k|v blocks), W_o
[1024, 1024], H=16 heads, head_dim=64, dense causal attention,
y = softmax(q k^T / 8, causal) v, out = y @ W_o.

Sharding: head-parallel over 8 NeuronCores (2 heads per core). Each core
computes q/k/v projections for its 2 heads, causal attention, and a partial
O-projection (its 128 attention-output columns against its 128 rows of W_o).
The host sums the 8 partial outputs.

On-core dataflow (bf16 into the PE, f32 accumulation in PSUM):
  - xT [128, (r d c)] arrives pre-transposed and pre-blocked from the host
    (column block r, contraction chunk d), so the r=0 quarter lands first
    and projections start before the rest of x arrives.
  - all weights + mask + identity arrive in ONE dram tensor w_all
    [128, 4352] = wq|wk|wv|wo|mask|ident (one DMA issue instead of six).
  - qT/kT/vT [128, T] = W.T @ xT (2 heads stacked on partitions); v is then
    materialized in [tk, hd] layout via PE transposes of vT 128x128 blocks
    (bf16 transpose datapath), evacuated by GpSimd into v_sb with a
    constant-1 column prepended per head ([1 | v_h]), so the attention-V
    matmul also accumulates the softmax denominator at partition 0.
  - attention is computed transposed: S_T [tk, tq] = kT-tile.T @ qT-tile,
    both heads concurrently via PE tiling (K=64 row halves), P_T = exp(S/8)
    in one ACT op per (tk, tq-block) covering both heads, causal mask
    applied on diagonal 128x128 blocks by a DVE multiply; fully-masked
    blocks skipped and both heads column-trimmed on diagonal blocks.
  - den/numer_T: [65, tq] = [1 | v_h].T @ P_T per head (den at row 0).
  - normalize: reciprocal_approx_fast directly on the PSUM den row (base
    0), cast bf16, broadcast to 128 partitions with a K=1 PE matmul, one
    DVE multiply per head writes normalized att rows.
  - y_partial [T, D] = att.T.T @ wo_rows; PSUM evacuated by DVE (cols
    0:512) and GpSimd (cols 512:1024), DMA'd bf16 on the Sync engine;
    summed on the host.

Engine budget: ACT does exp only (its table is preloaded during the input
DMA); DVE does casts/mask/normalize + half the O-proj evac; GpSimd does the
v scatter + the other half of the O-proj evac; Sync issues all DMAs.

Scheduling: the PE has a p-state ramp (full 2.4 GHz only after ~3us of
continuous execution), so the emission order keeps the PE dense:
  - warm-up matmuls on memset scratch cover the input-DMA wait;
  - the attention i-loop is software-pipelined: S(i+1) is emitted before
    AV(i); projection/O-projection work is drained quota-wise inside the
    i-loop to fill PE slack under the ACT exp stream;
  - O-proj tiles are spread across round boundaries so the post-round-3
    tail is only normalize(3) + 4 tiles.
"""

from contextlib import ExitStack

import numpy as np
import ml_dtypes

import concourse.bacc as bacc
import concourse.mybir as mybir
import concourse.tile as tile

BF16 = ml_dtypes.bfloat16
T = 2048
D = 1024
HD = 64
N_CORES = 8
KD = D // 128          # 8 contraction chunks for projections
NT128 = T // 128       # 16
NT512 = T // 512       # 4
VS = 256               # v_sb per-tile stride: [1*64 | v_h0(64) | 1*64 | v_h1(64)]
SCALE = 1.0 / 8.0      # 1/sqrt(64)
WARM_N = 36            # p-state warm-up matmuls while input DMA is in flight

# w_all column offsets
WQ0, WK0, WV0, WO0, MSK0, ID0 = 0, D, 2 * D, 3 * D, 4 * D, 4 * D + 128
WCOLS = 4 * D + 256

F32 = mybir.dt.float32
BF = mybir.dt.bfloat16

_SENTINEL = object()


class _Work:
    """Wraps an emission generator that yields False mid-segment and True at
    segment boundaries (points where every pool accumulation it opened is
    closed, so other users of the same pools may allocate)."""

    def __init__(self, gen):
        self.gen = gen
        self.at_boundary = True
        self.done = False

    def step(self):
        r = next(self.gen, _SENTINEL)
        if r is _SENTINEL:
            self.done = True
            self.at_boundary = True
            return False
        self.at_boundary = bool(r)
        return True

    def drain_to_boundary(self):
        while not (self.at_boundary or self.done):
            self.step()

    def drain_all(self):
        while not self.done:
            self.step()


def _kernel(tc, y, xT, w_all, dbg=None):
    nc = tc.nc
    Exp = mybir.ActivationFunctionType.Exp

    with ExitStack() as ctx:
        persist = ctx.enter_context(tc.tile_pool(name="persist", bufs=1))
        ps_mm = ctx.enter_context(tc.tile_pool(name="ps_mm", bufs=2, space="PSUM"))
        ps_s = ctx.enter_context(tc.tile_pool(name="ps_s", bufs=2, space="PSUM"))
        ps_av = ctx.enter_context(tc.tile_pool(name="ps_av", bufs=1, space="PSUM"))
        pool_p = ctx.enter_context(tc.tile_pool(name="pool_p", bufs=5))
        pool_r = ctx.enter_context(tc.tile_pool(name="pool_r", bufs=2))
        pool_y = ctx.enter_context(tc.tile_pool(name="pool_y", bufs=6))

        w_sb = persist.tile([128, WCOLS], BF, tag="w")
        xT_sb = persist.tile([128, KD * T], BF, tag="xT")  # block (r,d) at (r*8+d)*512

        # ---- p-state warm-up scratch.
        warm_sb = persist.tile([128, 640], BF, tag="warm")
        nc.vector.memset(warm_sb[:], 0.5)

        # ---- input DMA: qkv weights first, then each xT quarter striped
        # across the three issue queues IN QUARTER ORDER (sync carries the
        # w tensors, so its stripe is smaller), so the DMA engines finish
        # r0 before starting r1 and projections start early.
        nc.sync.dma_start(w_sb[:, 0:3 * D], w_all[:, 0:3 * D])
        act_warm = persist.tile([1, 8], F32, tag="actwarm")
        for r in range(4):
            q0 = r * 4096
            nc.scalar.dma_start(xT_sb[:, q0:q0 + 1792], xT[:, q0:q0 + 1792])
            nc.gpsimd.dma_start(
                xT_sb[:, q0 + 1792:q0 + 3584], xT[:, q0 + 1792:q0 + 3584])
            nc.sync.dma_start(
                xT_sb[:, q0 + 3584:q0 + 4096], xT[:, q0 + 3584:q0 + 4096])
            if r == 0:
                # wo|mask|ident after the r0 stripes (needed later than wqkv)
                nc.sync.dma_start(w_sb[:, 3 * D:WCOLS], w_all[:, 3 * D:WCOLS])
                # preload the ACT exp table during the DMA wait (the first
                # activation otherwise pays ~1.3us of table load on the
                # attention critical path)
                nc.scalar.activation(
                    act_warm[:], warm_sb[0:1, 0:8], Exp, scale=SCALE)

        qT_sb = persist.tile([128, T], BF, tag="qT")   # partitions 0-63 head0, 64-127 head1
        kT_sb = persist.tile([128, T], BF, tag="kT")
        vT_sb = persist.tile([128, T], BF, tag="vT")
        v_sb = persist.tile([128, NT128 * VS], BF, tag="v")
        v_cols = v_sb[:].rearrange("p (t s) -> p t s", s=VS)
        nc.vector.memset(v_cols[:, :, 0:64], 1.0)      # den-broadcast columns
        nc.vector.memset(v_cols[:, :, 128:192], 1.0)
        att_sb = persist.tile([128, T], BF, tag="att")  # normalized numer_T

        wq_sb = w_sb[:, WQ0:WQ0 + D]
        wk_sb = w_sb[:, WK0:WK0 + D]
        wv_sb = w_sb[:, WV0:WV0 + D]
        wo_sb = w_sb[:, WO0:WO0 + D]
        mask_sb = w_sb[:, MSK0:MSK0 + 128]
        id_sb = w_sb[:, ID0:ID0 + 128]

        def proj_gen(r, s0_out=None):
            """QKV projections for column-block r: 24 N=512 matmuls + 3 casts
            + 4 transposes (+ DVE scatters). Yields True when the open
            ps_mm segment has been closed. s0_out: emit round-0's first
            S-pair as soon as q/k are cast (before the v transposes)."""
            for wi, (w_part, dst) in enumerate(
                    ((wq_sb, qT_sb), (wk_sb, kT_sb), (wv_sb, vT_sb))):
                if wi == 2 and s0_out is not None:
                    s0_out.append(S_pair(0, 0))
                    yield True
                ps = ps_mm.tile([128, 512], F32, tag="mm")
                for d in range(KD):
                    nc.tensor.matmul(
                        ps[:],
                        lhsT=w_part[:, d * 128:(d + 1) * 128],
                        rhs=xT_sb[:, (r * KD + d) * 512:(r * KD + d + 1) * 512],
                        start=(d == 0), stop=(d == KD - 1),
                    )
                    yield False
                nc.vector.tensor_copy(dst[:, r * 512:(r + 1) * 512], ps[:])
                yield True
            for t in range(4 * r, 4 * r + 4):
                ps_t = ps_mm.tile([128, 128], BF, tag="mm")
                nc.tensor.transpose(
                    ps_t[:], vT_sb[:, t * 128:(t + 1) * 128], id_sb)
                yield False
                dst = v_cols[:, t, :].rearrange(
                    "p (g q) -> p g q", q=128)[:, :, 64:128]
                src = ps_t[:].rearrange("p (g q) -> p g q", q=64)
                nc.vector.tensor_copy(dst, src)
                yield True

        def oproj_gen(tiles, dve_only=False, act_evac=False):
            """O-projection rows for the given T-chunk indices; evacuation
            split DVE (cols 0:512) / ACT (cols 512:1024) so neither engine
            builds a backlog that would delay the round-boundary normalize.
            dve_only keeps evacs off ACT when the round is exp-paced;
            act_evac puts both on ACT (tail: DVE runs normalize(3))."""
            for t in tiles:
                y_sb = pool_y.tile([128, 1024], BF, tag="y")
                ps0 = ps_mm.tile([128, 512], F32, tag="mm")
                nc.tensor.matmul(
                    ps0[:], lhsT=att_sb[:, t * 128:(t + 1) * 128],
                    rhs=wo_sb[:, 0:512], start=True, stop=True,
                )
                yield False
                ps1 = ps_mm.tile([128, 512], F32, tag="mm")
                nc.tensor.matmul(
                    ps1[:], lhsT=att_sb[:, t * 128:(t + 1) * 128],
                    rhs=wo_sb[:, 512:1024], start=True, stop=True,
                )
                yield False
                if act_evac:
                    nc.scalar.copy(y_sb[:, 0:512], ps0[:])
                else:
                    nc.vector.tensor_copy(y_sb[:, 0:512], ps0[:])
                yield True
                if dve_only:
                    nc.vector.tensor_copy(y_sb[:, 512:1024], ps1[:])
                else:
                    nc.scalar.copy(y_sb[:, 512:1024], ps1[:])
                yield True
                nc.sync.dma_start(y[t * 128:(t + 1) * 128, :], y_sb[:])
                yield True

        def oproj_tail_gen(tiles):
            """Tail O-projection: the S-pipeline PSUM banks are free, so use
            [128,1024] ps_s tiles (both matmuls in flight per tile, no
            ps_mm round-trip). The first half of the tiles evacuates on ACT
            alone (DVE is still running normalize(3)'s second chunk); the
            rest split DVE/ACT. Each half DMAs out as soon as it lands."""
            for idx, t in enumerate(tiles):
                y_sb = pool_y.tile([128, 1024], BF, tag="y")
                ps = ps_s.tile([128, 1024], F32, tag="s")
                nc.tensor.matmul(
                    ps[:, 0:512], lhsT=att_sb[:, t * 128:(t + 1) * 128],
                    rhs=wo_sb[:, 0:512], start=True, stop=True,
                )
                yield False
                nc.tensor.matmul(
                    ps[:, 512:1024], lhsT=att_sb[:, t * 128:(t + 1) * 128],
                    rhs=wo_sb[:, 512:1024], start=True, stop=True,
                )
                yield False
                if idx == 0:
                    nc.scalar.copy(y_sb[:, 0:512], ps[:, 0:512])
                else:
                    nc.vector.tensor_copy(y_sb[:, 0:512], ps[:, 0:512])
                yield True
                nc.scalar.copy(y_sb[:, 512:1024], ps[:, 512:1024])
                yield True
                eng = nc.gpsimd if t % 2 else nc.sync
                eng.dma_start(y[t * 128:(t + 1) * 128, :], y_sb[:])
                yield True

        def S_pair(j, i):
            """S_T block matmuls for (tq round j, tk block i): both heads
            concurrently on disjoint PE row-halves (K=64 tile positions)."""
            m = i - 4 * j
            off = 128 * m if m > 0 else 0
            s_pair = ps_s.tile([128, 1024], F32, tag="s")
            nc.tensor.matmul(
                s_pair[:, off:512],
                lhsT=kT_sb[0:64, i * 128:(i + 1) * 128],
                rhs=qT_sb[0:64, j * 512 + off:(j + 1) * 512],
                start=True, stop=True, tile_position=(0, 0),
            )
            nc.tensor.matmul(
                s_pair[:, 512:1024 - off],
                lhsT=kT_sb[64:128, i * 128:(i + 1) * 128],
                rhs=qT_sb[64:128, j * 512 + off:(j + 1) * 512],
                start=True, stop=True, tile_position=(64, 0),
            )
            return s_pair

        def attn_round(j, work, total, reserve, s0=None):
            """Causal attention for tq block j, software-pipelined; spreads
            ~(total - reserve) interleave items evenly over the i-loop so the
            PE stays ahead of ACT on every iteration. s0 is the pre-emitted
            S-pair for i=0 (emitted before the previous round boundary's
            PSUM-gated filler, so the PE never goes idle at the boundary)."""
            n_i = 4 * j + 4
            remaining = max(0, total - reserve)
            avden = ps_av.tile([128, 1024], F32, tag="avden")
            s_tiles = [None] * n_i

            s_tiles[0] = s0 if s0 is not None else S_pair(j, 0)
            for i in range(n_i):
                if i + 1 < n_i:
                    s_tiles[i + 1] = S_pair(j, i + 1)
                m = i - 4 * j
                off = 128 * m if m > 0 else 0
                ncol = 512 - off
                first, last = (i == 0), (i == n_i - 1)
                s_pair = s_tiles[i]
                s_tiles[i] = None
                p_sb = pool_p.tile([128, 1024], BF, tag="p")
                nc.scalar.activation(
                    p_sb[:, off:512 + ncol], s_pair[:, off:512 + ncol], Exp,
                    scale=SCALE,
                )
                if m >= 0:  # causal mask on the 128x128 diagonal sub-blocks
                    nc.vector.tensor_mul(
                        p_sb[:, off:off + 128], p_sb[:, off:off + 128], mask_sb,
                    )
                    nc.vector.tensor_mul(
                        p_sb[:, 512:640], p_sb[:, 512:640], mask_sb,
                    )
                # interleave filler BEFORE the AV pair: the PE chews on it
                # while ACT finishes exp(i) (and the pool the mask), instead
                # of idling in-order behind the AV's data dependency.
                k = -(-remaining // (n_i - i))  # ceil: spread evenly
                remaining -= k
                for _ in range(k):
                    if not work.step():
                        break
                nc.tensor.matmul(
                    avden[:, off:512],
                    lhsT=v_sb[:, VS * i: VS * i + 128],
                    rhs=p_sb[:, off:512],
                    start=first, stop=last,
                )
                nc.tensor.matmul(
                    avden[:, 512 + off:1024],
                    lhsT=v_sb[:, VS * i + 128: VS * i + 256],
                    rhs=p_sb[:, 512:512 + ncol],
                    start=first, stop=last,
                )
            return avden

        def normalize(j, avden, split=1):
            """The AV matmul already broadcast the denominator to partitions
            0-63 (64 ones columns in the v lhsT); recip it straight from
            PSUM (base 0) and one DVE mul per head writes normalized att.
            split>1 processes tq column chunks separately so downstream
            O-proj tiles unlock as soon as their chunk is normalized."""
            w = 512 // split
            for c in range(split):
                recs = []
                for h in range(2):
                    rec = pool_r.tile([64, w], F32, tag=f"rec{c}")
                    nc.vector.reciprocal_approx_fast(
                        rec[:], avden[0:64, h * 512 + c * w:h * 512 + (c + 1) * w])
                    recs.append(rec)
                for h in range(2):
                    nc.vector.tensor_mul(
                        att_sb[h * 64:(h + 1) * 64,
                               j * 512 + c * w:j * 512 + (c + 1) * w],
                        avden[64:128, h * 512 + c * w:h * 512 + (c + 1) * w],
                        recs[h][:],
                    )

        # warm-up stream, then round 0 projections (PE hot when xT lands)
        warm_ps = ps_s.tile([128, 1024], F32, tag="s")
        for _ in range(WARM_N):
            nc.tensor.matmul(
                warm_ps[:, 0:512], lhsT=warm_sb[:, 0:128], rhs=warm_sb[:, 128:640],
                start=True, stop=True,
            )
        s0h = []
        for _ in proj_gen(0, s0_out=s0h):
            pass

        # r=0: attn(0) ⟂ proj(1); normalize(0); proj(1) rest; oproj t 0-1
        # r=1: attn(1) ⟂ proj(2); normalize(1); proj(2) rest; oproj t 2-3
        # r=2: attn(2) ⟂ proj(3); normalize(2); proj(3) rest; oproj t 4-5
        # r=3: attn(3) ⟂ oproj t 6-7 (small: attn3 is ACT-paced);
        #      normalize(3) with oproj t 8-11 matmuls feeding the PE while
        #      the DVE runs the recip/mul chain; then the ps_s-based tail.
        work = _Work(proj_gen(1))
        s0 = s0h[0]
        for r in range(NT512):
            if r < 3:
                avden = attn_round(r, work, 35, 6, s0=s0)
            else:
                avden = attn_round(r, work, 35, 4, s0=s0)
            # pre-emit the next round's first S-pair so the PE has
            # dependency-free work queued ahead of the PSUM-slot-gated
            # boundary fillers (in-order head-of-line blocking otherwise
            # idles the PE and drops its p-state clock)
            s0 = S_pair(r + 1, 0) if r + 1 < NT512 else None
            # normalize only touches pool_r/avden/att, so emit it before the
            # leftover work: its DVE ops jump ahead of the remaining evacs
            normalize(r, avden, split=(2 if r == 3 else 1))
            work.drain_all()
            if r == 0:
                _Work(oproj_gen([0, 1])).drain_all()
                work = _Work(proj_gen(2))
            elif r == 1:
                work = _Work(proj_gen(3))
            elif r == 2:
                work = _Work(
                    oproj_gen([2, 3, 4, 5, 6, 7, 8], dve_only=True))
            else:
                _Work(oproj_gen([9, 10, 11], act_evac=True)).drain_all()
                _Work(oproj_tail_gen([12, 13, 14, 15])).drain_all()

        if dbg is not None:
            for name, sb in (("qT", qT_sb), ("kT", kT_sb), ("vT", vT_sb),
                             ("att", att_sb)):
                nc.sync.dma_start(dbg[name][:], sb[:])
            nc.sync.dma_start(dbg["v"][:], v_sb[:])
            nc.sync.dma_start(dbg["xT"][:], xT_sb[:])
            nc.sync.dma_start(dbg["w"][:], w_sb[:])


def _build_program(debug_dumps=False):
    nc = bacc.Bacc("TRN2", debug=False, num_devices=N_CORES)
    xT = nc.dram_tensor("xT", [128, KD * T], BF, kind="ExternalInput").ap()
    w_all = nc.dram_tensor("w_all", [128, WCOLS], BF, kind="ExternalInput").ap()
    y = nc.dram_tensor("y", [T, D], BF, kind="ExternalOutput").ap()
    dbg = None
    if debug_dumps:
        dbg = {
            name: nc.dram_tensor(f"dbg_{name}", [128, T], BF, kind="ExternalOutput").ap()
            for name in ("qT", "kT", "vT", "att")
        }
        dbg["v"] = nc.dram_tensor("dbg_v", [128, NT128 * VS], BF, kind="ExternalOutput").ap()
        dbg["xT"] = nc.dram_tensor("dbg_xT", [128, KD * T], BF, kind="ExternalOutput").ap()
        dbg["w"] = nc.dram_tensor("dbg_w", [128, WCOLS], BF, kind="ExternalOutput").ap()

    with tile.TileContext(nc) as tc:
        _kernel(tc, y, xT, w_all, dbg=dbg)
    nc.compile()
    return nc


_NC = None


def _get_program():
    global _NC
    if _NC is None:
        _NC = _build_program()
    return _NC


def _rearrange_w(w_cols):
    """[1024, 128] f32 slice of W_qkv -> [128, 1024] bf16 with d-chunk d at
    cols [d*128, (d+1)*128): out[p, d*128 + m] = w_cols[d*128 + p, m]."""
    return np.ascontiguousarray(
        w_cols.reshape(KD, 128, 128).transpose(1, 0, 2).reshape(128, KD * 128)
    ).astype(BF16)


def make_in_maps(x, W_qkv, W_o):
    x2 = np.asarray(x, dtype=np.float32).reshape(T, D)
    W_qkv = np.asarray(W_qkv, dtype=np.float32)
    W_o = np.asarray(W_o, dtype=np.float32)

    # xT blocked [p, r, d, c]: col (r*8+d)*512 + c holds x[r*512+c, d*128+p]
    xT_bf = np.ascontiguousarray(
        x2.T.reshape(KD, 128, NT512, 512).transpose(1, 2, 0, 3).reshape(128, KD * T)
    ).astype(BF16)
    mask = np.triu(np.ones((128, 128), dtype=np.float32)).astype(BF16)
    ident = np.eye(128, dtype=np.float32).astype(BF16)

    in_maps = []
    for c in range(N_CORES):
        cs = slice(2 * c * HD, 2 * c * HD + 128)
        w_all = np.concatenate([
            _rearrange_w(W_qkv[:, 0 * D:1 * D][:, cs]),
            _rearrange_w(W_qkv[:, 1 * D:2 * D][:, cs]),
            _rearrange_w(W_qkv[:, 2 * D:3 * D][:, cs]),
            np.ascontiguousarray(W_o[c * 128:(c + 1) * 128, :]).astype(BF16),
            mask,
            ident,
        ], axis=1)
        in_maps.append({"xT": xT_bf, "w_all": w_all})
    return in_maps


def combine_outputs(results):
    y_full = np.zeros((T, D), dtype=np.float32)
    for c in range(N_CORES):
        y_full += results[c]["y"].astype(np.float32)
    return y_full.reshape(1, T, D)


def kernel(x, W_qkv, W_o):
    from concourse.bass_utils import run_bass_kernel_spmd

    nc = _get_program()
    in_maps = make_in_maps(x, W_qkv, W_o)
    res = run_bass_kernel_spmd(nc, in_maps, core_ids=list(range(N_CORES)))
    return combine_outputs(res.results)
